# revision 71
# baseline (speedup 1.0000x reference)
"""Trainium2 Bass kernel for nn_ColumnStep (scatter_memory).

Contract: kernel(**inputs) takes FULL unsharded inputs (numpy-convertible),
returns the FULL (B, T, V) float32 output.

Sharding: 8 cores = B(2) x T-query-chunks(4); parameters replicated. Host
does only the vocab gather / zero-scatter and layout prep.

Structure (v2):
- decay = sigmoid(decay_logit) truncates the anti-causal attention to a
  ~(nd-1)*128-token future window (first omitted diagonal <= 2.5e-3,
  measured well inside the rel-err budget); each core loads a
  (512 + 128*(nd-1)) column window and the decay weights collapse to
  one Toeplitz band.
- LAZY RMSNORM: q/k/v projections run on the RAW gathered columns straight
  off the DMA; the first rmsnorm is folded in afterwards as
  ws[j,i] = sc[j,i] * (1/(mean_j+eps)) * (decay/sqrt(k))  (per-j recip via
  column-sum matmuls) and retr_sb = retr * rsqrt(mean_i+eps) (row rsqrt,
  broadcast by matmul). No normalized activations are ever materialized
  for the first norm, and nothing on the PE critical path waits for it.
- ONE act table: rsqrt = Exp(-0.5*Ln(.)) keeps every pre-MLP activation in
  the ln/exp table; the single switch to the gelu table happens while the
  PE is busy with branch matmuls.
- All DMA'd tensors are bf16 (weights stationary-side, window data at
  full matmul rate as f32-free moving operands are not needed); this
  halves the serialized DMA-engine occupancy. Small f32 constants ride a
  tiny f32 DMA; other constants are memset on device.
"""

import sys

for _p in ("/opt/trn_rl_repo", "/root/.axon_site/_ro/trn_rl_repo"):
    if _p not in sys.path:
        sys.path.append(_p)

import math

import ml_dtypes
import numpy as np

import concourse.bass as bass  # noqa: F401  (registers engine mixins)
import concourse.mybir as mybir
from concourse import bacc, tile
from concourse.bass_utils import run_bass_kernel_spmd

F32 = mybir.dt.float32
F32R = mybir.dt.float32r
BF16 = mybir.dt.bfloat16
AF = mybir.ActivationFunctionType
OP = mybir.AluOpType
BF = ml_dtypes.bfloat16

# Problem shape (hardcoded per spec)
V, K, B, T, NB, INNER = 32000, 256, 2, 2048, 4, 128
EPS = 1.1920929e-07
P = 128          # partitions
QF = T // 4      # 512 query rows per core
NQ = QF // P     # 4 query tiles per core
KT = K // P      # 2 tiles along the k=256 dim

# pack offsets (bf16 columns per partition); order = DMA priority
O_W = 0                        # Wk,Wq,Wv,Wo  (2048; 512 each)
O_GW = O_W + 4 * KT * K        # gate_W       (8)
O_BD = O_GW + KT * NB          # branch_down  (1024)
O_BU = O_BD + NB * KT * INNER  # branch_up    (1024)
PK = O_BU + NB * K             # 4112 total
WK, WQ, WV, WO = 0, 1, 2, 3

_prog_cache = {}


def _build_program(c_mem, s_out, nd, const_gates):
    """SPMD Bass/Tile program. nd = number of 128-wide j-tile diagonals
    (1 own + nd-1 future) each query tile attends to. const_gates: gate_W
    is all-zero (the spec fill), so the softmax gates are per-branch
    constants already folded into the e-selector by the host."""
    nc = bacc.Bacc("TRN2", target_bir_lowering=False, debug=False, num_devices=8)

    WIN = QF + P * (nd - 1)   # key/value window columns per core
    NJ = NQ + nd - 1          # local j tiles
    MW = P * nd               # decay master columns

    gw_d = nc.dram_tensor("gw", [P, KT, WIN], BF16, kind="ExternalInput")
    m_d = nc.dram_tensor("m", [P, MW], BF16, kind="ExternalInput")
    pack_d = nc.dram_tensor("pack", [P, PK], BF16, kind="ExternalInput")
    # small f32 constants: [ mlp_bias | gate_b(rows 0..NB-1) ]
    small_d = nc.dram_tensor("small", [P, 2], F32, kind="ExternalInput")
    if not const_gates:
        esel_d = nc.dram_tensor("esel", [NB, NB * P], F32R, kind="ExternalInput")
    o_d = nc.dram_tensor("o", [NQ, P, K], BF16, kind="ExternalOutput")

    # rms chunks over the window: [(start, end), ...] in <=512 steps
    chunks = [(c, min(c + 512, WIN)) for c in range(0, WIN, 512)]

    with tile.TileContext(nc) as tc:
        with (
            nc.allow_low_precision(reason="bf16 operands validated by rel-err test"),
            tc.tile_pool(name="const", bufs=1) as cp,
            tc.tile_pool(name="persist", bufs=1) as pp,
            tc.tile_pool(name="work", bufs=3) as wp,
            tc.tile_pool(name="stat", bufs=4) as sp,
            tc.tile_pool(name="psA", bufs=3, space="PSUM") as psA,
            tc.tile_pool(name="psS", bufs=2, space="PSUM") as psS,
            tc.tile_pool(name="psR", bufs=1, space="PSUM") as psR,
            tc.tile_pool(name="psN", bufs=1, space="PSUM") as psN,
        ):
            # ---- constants / parameters ----
            pack_t = cp.tile([P, PK], BF16, tag="pack")
            w_all = pack_t[:, O_W:O_GW].rearrange("p (w t k) -> p w t k", w=4, t=KT)
            gw_wt = pack_t[:, O_GW:O_BD].rearrange("p (t n) -> p t n", t=KT)
            bd_t = pack_t[:, O_BD:O_BU].rearrange("p (n t h) -> p n t h", n=NB, t=KT)
            bu_t = pack_t[:, O_BU:PK].rearrange("p (n k) -> p n k", n=NB)
            small_t = cp.tile([P, 2], F32, tag="small")
            biash_t = small_t[:, 0:1]
            gatebT = small_t[0:NB, 1:2]
            m_t = cp.tile([P, MW], BF16, tag="mmat")
            eps1_t = cp.tile([1, 1], F32, tag="eps1")
            warm_t = cp.tile([1, 1], F32, tag="warm")
            ones_f = cp.tile([P, 1], F32, tag="onesc")
            ones_col = ones_f[:].bitcast(F32R)
            ones_bf = cp.tile([P, 1], BF16, tag="onesb16")
            onesr_f = cp.tile([1, P], F32, tag="onesr")
            onesr_t = onesr_f[:].bitcast(F32R)
            if not const_gates:
                e_t = cp.tile([NB, NB * P], F32R, tag="esel")
            gw_sb = cp.tile([P, KT, WIN], BF16, tag="gwin")

            kkb = [pp.tile([P, WIN], BF16, tag=f"kkb{i}", name=f"kkb{i}") for i in range(KT)]

            # ---- act-table warm-up: first table = ln/exp set; the single
            # switch to the gelu set happens mid-epilogue under PE shadow ----
            nc.vector.memset(eps1_t[:], EPS)
            nc.vector.memset(warm_t[:], 0.0)
            # explicit load of natural_log_exp_and_others (act_info idx 6):
            # covers Square/Copy/Ln/Exp so the naive per-function chooser
            # never ping-pongs between the ln-only and exp-only tables
            nc.scalar.add_instruction(mybir.InstLoadActFuncSet(
                name=nc.get_next_instruction_name(), ins=[], outs=[],
                act_func_set_id=6))
            nc.scalar.activation(warm_t[:], warm_t[:], AF.Exp)

            # ---- DMAs in priority order (first-use first). m+small go via
            # the Pool SWDGE path, which bypasses the shared HWDGE queue ----
            nc.sync.dma_start(pack_t[:, 0:512], pack_d[:, 0:512])          # Wk
            nc.sync.dma_start(gw_sb[:, :, 0:512], gw_d[:, :, 0:512])
            nc.gpsimd.dma_start(m_t[:], m_d[:])
            nc.gpsimd.dma_start(small_t[:], small_d[:])
            nc.sync.dma_start(pack_t[:, 512:1024], pack_d[:, 512:1024])    # Wq
            nc.sync.dma_start(pack_t[:, 1024:1536], pack_d[:, 1024:1536])  # Wv
            if WIN > 512:
                nc.sync.dma_start(gw_sb[:, :, 512:WIN], gw_d[:, :, 512:WIN])
            nc.sync.dma_start(pack_t[:, 1536:PK], pack_d[:, 1536:PK])      # Wo..

            # ---- device-built constants (tiny, off critical path) ----
            nc.vector.memset(ones_f[:], 1.0)
            nc.scalar.copy(ones_bf[:], ones_f[:])
            nc.vector.memset(onesr_f[:], 1.0)
            if not const_gates:
                nc.gpsimd.dma_start(e_t[:], esel_d[:])

            # ---- persistent intermediates ----

            qb = [pp.tile([P, QF], BF16, tag=f"qb{i}", name=f"qb{i}") for i in range(KT)]
            vvb = [pp.tile([P, K], BF16, tag=f"vvb{j}", name=f"vvb{j}") for j in range(NJ)]
            sq = [pp.tile([P, KT, 512], BF16, tag=f"sq{c}", name=f"sq{c}")
                  for c in range(len(chunks))]
            rinv2c = pp.tile([P, NJ + 2], F32, tag="rinv2c")
            retr_sb = [pp.tile([P, QF], BF16, tag=f"retr{i}", name=f"retr{i}") for i in range(KT)]
            g2T = [pp.tile([P, QF], F32, tag=f"g2T{i}", name=f"g2T{i}") for i in range(KT)]
            gn2T = [pp.tile([P, QF], BF16, tag=f"gn2T{i}", name=f"gn2T{i}") for i in range(KT)]
            hgel = [pp.tile([P, QF], BF16, tag=f"hgel{n}", name=f"hgel{n}") for n in range(NB)]
            if not const_gates:
                hg = [pp.tile([P, QF], BF16, tag=f"hg{n}", name=f"hg{n}") for n in range(NB)]
            exr = pp.tile([NB, QF], F32R, tag="exr")
            rt2 = pp.tile([1, QF], F32R, tag="rt2")
            rcT = [sp.tile([P, 1], F32, tag=f"rcT{q}", name=f"rcT{q}") for q in range(NQ)]
            o_sb = pp.tile([P, NQ, K], BF16, tag="osb")

            # cols: [kt0-qt(2h) | kt0-qt(2h+1) | kt1-qt(2h) | kt1-qt(2h+1)]
            retr_ps = [psR.tile([P, 2 * KT * P], F32, tag=f"rps{h}",
                                name=f"rps{h}") for h in range(2)]
            gexp_insts = []
            bcq_ref = []

            # ---- phase B helpers ----
            copy_engines = [nc.scalar, nc.vector]
            cp_i = [0]

            def psum_copy(dst, src):
                eng = copy_engines[cp_i[0] % 2]
                cp_i[0] += 1
                if eng is nc.scalar:
                    eng.copy(dst, src)
                else:
                    eng.tensor_copy(dst, src)

            def psum_copy2(dst, src, w):
                # split a wide PSUM->SBUF copy across Act+DVE so the psA
                # rotation unblocks at matmul speed
                h = w // 2
                nc.scalar.copy(dst[:, 0:h], src[:, 0:h])
                nc.vector.tensor_copy(dst[:, h:w], src[:, h:w])

            def attention(jt):
                lo = max(0, jt - (nd - 1))
                hi = min(NQ - 1, jt)
                ib = lo * P
                wdt = (hi - lo + 1) * P
                ms = P * (nd - 1) - P * min(jt, nd - 1)
                sc = psS.tile([P, 512], F32, tag="sc")
                for ki in range(KT):
                    nc.tensor.matmul(
                        sc[:, :wdt], kkb[ki][:, jt * P:(jt + 1) * P],
                        qb[ki][:, ib:ib + wdt],
                        start=(ki == 0), stop=(ki == KT - 1))
                ws = wp.tile([P, 512], BF16, tag="ws")
                # ws = sc * (1/(mean_j+eps)) * (s_qk-scaled decay master).
                # last two j tiles go Act+Pool to keep DVE free for the
                # epilogue front
                nc.vector.scalar_tensor_tensor(
                    ws[:, :wdt], sc[:, :wdt], rinv2c[:, jt:jt + 1],
                    m_t[:, ms:ms + wdt], op0=OP.mult, op1=OP.mult)
                for qt in range(lo, hi + 1):
                    off = qt * P - ib
                    for kt in range(KT):
                        c0_ = kt * 2 * P + (qt % 2) * P
                        nc.tensor.matmul(
                            retr_ps[qt // 2][:, c0_:c0_ + P],
                            vvb[jt][:, kt * P:(kt + 1) * P],
                            ws[:, off:off + P],
                            start=(jt == qt), stop=(jt == qt + nd - 1))

            # ---- phase B: raw projections + lazy-norm attention.
            # emission order = psA rotation priority: k(c0), q, k(c1)
            # first so every score input exists early, then v, then the
            # attention sweep ----
            for ci, (c0, c1) in enumerate(chunks):
                w = c1 - c0
                nc.scalar.square(sq[ci][:, 0, :w], gw_sb[:, 0, c0:c1])
                nc.gpsimd.tensor_mul(sq[ci][:, 1, :w], gw_sb[:, 1, c0:c1],
                                     gw_sb[:, 1, c0:c1])

            def kproj(ci, c0, c1):
                w = c1 - c0
                for ko in range(KT):
                    ps = psA.tile([P, 512], F32, tag="mm")
                    for ki in range(KT):
                        nc.tensor.matmul(
                            ps[:, :w], w_all[:, WK, ki, ko * P:(ko + 1) * P],
                            gw_sb[:, ki, c0:c1],
                            start=(ki == 0), stop=(ki == KT - 1))
                    if w > 256:
                        psum_copy2(kkb[ko][:, c0:c1], ps, w)
                    else:
                        psum_copy(kkb[ko][:, c0:c1], ps[:, :w])

            kproj(0, *chunks[0])
            for ko in range(KT):
                ps = psA.tile([P, 512], F32, tag="mm")
                for ki in range(KT):
                    nc.tensor.matmul(
                        ps[:], w_all[:, WQ, ki, ko * P:(ko + 1) * P],
                        gw_sb[:, ki, 0:QF],
                        start=(ki == 0), stop=(ki == KT - 1))
                psum_copy2(qb[ko][:], ps, QF)
            for ci in range(1, len(chunks)):
                kproj(ci, *chunks[ci])

            # per-j-tile column sums -> 1/(mean+eps); query-row rsqrt chain
            for ci, (c0, c1) in enumerate(chunks):
                jlo, jhi = c0 // P, min(c1 // P, NJ)
                colps = psN.tile([P, 8], F32, tag="nrm", name=f"colps{ci}")
                for jt in range(jlo, jhi):
                    for ki in range(KT):
                        nc.tensor.matmul(
                            colps[:, jt - jlo:jt - jlo + 1],
                            sq[ci][:, ki, jt * P - c0:(jt + 1) * P - c0],
                            ones_bf[:],
                            start=(ki == 0), stop=(ki == KT - 1))
                mean_c = sp.tile([P, 8], F32, tag="meanc", name=f"meanc{ci}")
                nc.vector.tensor_scalar(mean_c[:, :jhi - jlo],
                                        colps[:, :jhi - jlo], 1.0 / K, EPS,
                                        op0=OP.mult, op1=OP.add)
                nc.vector.reciprocal(rinv2c[:, jlo:jhi], mean_c[:, :jhi - jlo])
                if ci == 0:
                    csq = psN.tile([1, QF], F32, tag="nrm", name="csq")
                    for ki in range(KT):
                        nc.tensor.matmul(csq[:], ones_bf[:], sq[0][:, ki, 0:QF],
                                         start=(ki == 0), stop=(ki == KT - 1))
                    lnr = sp.tile([1, QF], F32R, tag="lnr")
                    nc.scalar.activation(lnr[:], csq[:], AF.Ln,
                                         bias=eps1_t[:], scale=1.0 / K)
                    rtr = sp.tile([1, QF], F32R, tag="rtr")
                    nc.scalar.activation(rtr[:], lnr[:], AF.Exp, scale=-0.5)
                    bcq = pp.tile([P, QF], F32R, tag="bcqsb")
                    nc.gpsimd.partition_broadcast(bcq[:], rtr[:])
                    bcq_ref.append(bcq)

            # v projections
            for jt in range(NJ):
                ps = psA.tile([P, K], F32, tag="mm")
                for ki in range(KT):
                    nc.tensor.matmul(
                        ps[:], gw_sb[:, ki, jt * P:(jt + 1) * P], w_all[:, WV, ki, :],
                        start=(ki == 0), stop=(ki == KT - 1))
                psum_copy(vvb[jt][:], ps[:])
            for jt in range(NJ):
                attention(jt)

            # ---- epilogue, full-width (512): Wo -> residual -> rmsnorm
            # -> gates -> MLP -> out. One chain; each elementwise stage
            # splits ko0->DVE / ko1->Pool so the two k-halves overlap. The
            # single gelu-table switch sits right after the one gates-exp. ----
            csq2 = psN.tile([1, QF], F32, tag="nrm", name="csq2")
            bc2s = pp.tile([P, QF], F32R, tag="bc2sb")
            if not const_gates:
                gp = psN.tile([NB, QF], F32, tag="nrm", name="gp")
                smT = psS.tile([P, 4], F32, tag="sc", name="smT")
            bpt = [psS.tile([P, K], F32, tag="sc", name="bpq0"),
                   psS.tile([P, K], F32, tag="sc", name="bpq1"),
                   psR.tile([P, K], F32, tag="rps0", name="bpq2"),
                   psR.tile([P, K], F32, tag="rps1", name="bpq3")]

            bcq = bcq_ref[0]

            def epi_front(h):
                hc = slice(2 * h * P, 2 * (h + 1) * P)
                nc.vector.tensor_mul(retr_sb[0][:, hc],
                                     retr_ps[h][:, 0:2 * P], bcq[:, hc])
                nc.vector.tensor_mul(retr_sb[1][:, hc],
                                     retr_ps[h][:, 2 * P:4 * P], bcq[:, hc])
                for ko in range(KT):
                    ps = psA.tile([P, 2 * P], F32, tag="mm")
                    for ki in range(KT):
                        nc.tensor.matmul(
                            ps[:], w_all[:, WO, ki, ko * P:(ko + 1) * P],
                            retr_sb[ki][:, hc],
                            start=(ki == 0), stop=(ki == KT - 1))
                    nc.vector.scalar_tensor_tensor(
                        g2T[ko][:, hc], ps[:], c_mem, gw_sb[:, ko, hc],
                        op0=OP.mult, op1=OP.add)
                sq2 = wp.tile([P, KT, 2 * P], F32R, tag="sq2")
                nc.scalar.square(sq2[:, 0, :], g2T[0][:, hc])
                nc.vector.tensor_mul(sq2[:, 1, :], g2T[1][:, hc],
                                     g2T[1][:, hc])
                for ki in range(KT):
                    nc.tensor.matmul(csq2[:, hc], ones_col, sq2[:, ki, :],
                                     start=(ki == 0), stop=(ki == KT - 1))
                nc.scalar.activation(rt2[:, hc], csq2[:, hc], AF.Ln,
                                     bias=eps1_t[:], scale=1.0 / K)
                ei = nc.scalar.activation(rt2[:, hc], rt2[:, hc], AF.Exp,
                                          scale=-0.5)
                nc.gpsimd.partition_broadcast(bc2s[:, hc], rt2[:, hc])
                nc.vector.tensor_mul(gn2T[0][:, hc], g2T[0][:, hc],
                                     bc2s[:, hc])
                nc.gpsimd.tensor_mul(gn2T[1][:, hc], g2T[1][:, hc],
                                     bc2s[:, hc])
                return ei

            exp_insts = [epi_front(0), epi_front(1)]
            if not const_gates:
                for ki in range(KT):
                    nc.tensor.matmul(gp[:], gw_wt[:, ki, :], gn2T[ki][:],
                                     start=(ki == 0), stop=(ki == KT - 1))
                # exp(logits + gate_b): gate_b folded into the bias
                exp_insts.append(
                    nc.scalar.activation(exr[:], gp[:], AF.Exp, bias=gatebT))
                for qt in range(NQ):
                    nc.tensor.matmul(smT[:, qt:qt + 1],
                                     exr[:, qt * P:(qt + 1) * P],
                                     ones_col[0:NB], start=True, stop=True)
                    nc.vector.reciprocal(rcT[qt][:], smT[:, qt:qt + 1])

            # MLP: bd -> gelu -> (gate ->) up, full-width, gelu-paced
            for n in range(NB):
                hp = psA.tile([P, QF], F32, tag="mm")
                for ki in range(KT):
                    nc.tensor.matmul(
                        hp[:], bd_t[:, n, ki, :], gn2T[ki][:],
                        start=(ki == 0), stop=(ki == KT - 1))
                gi = nc.scalar.activation(hgel[n][:], hp[:], AF.Gelu,
                                          bias=biash_t)
                # keep the single gelu-table switch after the last
                # exp-family op in the scheduled act stream
                deps = bass.InstructionNameOrderedSet()
                for e in exp_insts:
                    deps.add(e.ins.name)
                gi.ins.add_nosync_dependencies_from(deps)
                if not const_gates:
                    gb = psA.tile([P, QF], F32, tag="mm")
                    nc.tensor.matmul(gb[:], e_t[:, n * P:(n + 1) * P],
                                     exr[:], start=True, stop=True)
                    nc.vector.tensor_mul(hg[n][:], hgel[n][:], gb[:])
            up_in = hgel if const_gates else hg
            for n in range(NB):
                for qt in range(NQ):
                    nc.tensor.matmul(
                        bpt[qt][:],
                        up_in[n][:, qt * P:(qt + 1) * P], bu_t[:, n, :],
                        start=(n == 0), stop=(n == NB - 1))
            for qt in range(NQ):
                src = bpt[qt][:]
                dst = o_sb[:, qt, :]
                if const_gates:
                    if qt % 2 == 0:
                        nc.scalar.copy(dst, src)
                    else:
                        nc.vector.tensor_copy(dst, src)
                elif qt % 2 == 0:
                    nc.scalar.activation(dst, src, AF.Copy,
                                         scale=rcT[qt][:])
                else:
                    nc.vector.tensor_scalar(dst, src,
                                            rcT[qt][:], None, op0=OP.mult)
                if qt % 2 == 1:
                    nc.sync.dma_start(
                        o_d[qt - 1:qt + 1].rearrange("q p k -> p q k"),
                        o_sb[:, qt - 1:qt + 1, :])

    nc.compile()
    return nc


def kernel(**inputs):
    x = np.asarray(inputs["x"], np.float32)
    Wq = np.asarray(inputs["Wq"], np.float32)
    Wk = np.asarray(inputs["Wk"], np.float32)
    Wv = np.asarray(inputs["Wv"], np.float32)
    Wo = np.asarray(inputs["Wo"], np.float32)
    decay_logit = np.float32(np.asarray(inputs["decay_logit"]).reshape(()))
    out_scale = np.float32(np.asarray(inputs["out_scale"]).reshape(()))
    mem_scale = np.float32(np.asarray(inputs["mem_scale"]).reshape(-1)[0])
    branch_down = np.asarray(inputs["branch_down"], np.float32)
    branch_up = np.asarray(inputs["branch_up"], np.float32)
    mlp_bias = np.asarray(inputs["mlp_bias"], np.float32)
    gate_W = np.asarray(inputs["gate_W"], np.float32)
    gate_b = np.asarray(inputs["gate_b"], np.float32)
    write_scale = np.float32(np.asarray(inputs["write_scale"]).reshape(()))
    read_idx = np.asarray(inputs["read_indices"]).astype(np.int64)
    write_idx = np.asarray(inputs["write_indices"]).astype(np.int64)

    # Host-side gather of the active vocab subspace (data movement only).
    g = np.take(x, read_idx, axis=2)  # (B, T, K)

    decay = float(1.0 / (1.0 + np.exp(-float(decay_logit))))
    # window depth: smallest nd with decay^(128*(nd-1)) <= 2.5e-3 (first
    # omitted diagonal's largest weight; validated vs the rel-err budget);
    # nd=2 minimum, 16 = full sequence
    if decay <= 0.0:
        nd = 2
    else:
        nd = max(2, 1 + int(math.ceil(math.log(2.5e-3) / math.log(decay) / 128.0)))
    nd = min(nd, 16)

    s_qk = float(1.0 / np.sqrt(np.float32(K)))
    c_mem = float(out_scale * mem_scale)
    s_base = float(write_scale * np.float32(1.0 / 16.0))
    const_gates = bool(np.all(gate_W == 0.0))

    key = (round(c_mem, 12), nd, const_gates)
    nc = _prog_cache.get(key)
    if nc is None:
        nc = _build_program(c_mem, [s_base] * NB, nd, const_gates)
        _prog_cache[key] = nc

    WIN = QF + P * (nd - 1)
    MW = P * nd

    # Replicated parameter pack (partition-first, bf16); order matches the
    # O_* offsets: [Wk,Wq | Wv,Wo | gate_W | bd | bu].
    wall = np.stack([Wk, Wq, Wv, Wo]).reshape(4, KT, P, K).transpose(2, 0, 1, 3)
    gwp = gate_W.reshape(KT, P, NB).transpose(1, 0, 2)
    bdall = branch_down.reshape(NB, KT, P, INNER).transpose(2, 0, 1, 3)
    if const_gates:
        # gates are softmax(gate_b) constants; fold gate*write_scale/sqrt(k)
        # into the up-projection weights
        eg = np.exp(gate_b - gate_b.max())
        buall = (branch_up * (s_base * (eg / eg.sum()))[:, None, None]
                 ).transpose(1, 0, 2)
    else:
        buall = branch_up.transpose(1, 0, 2)
    pack = np.concatenate([
        wall.reshape(P, -1), gwp.reshape(P, -1),
        bdall.reshape(P, -1), buall.reshape(P, -1)], axis=1).astype(BF)
    small = np.zeros((P, 2), np.float32)
    small[:, 0] = mlp_bias
    small[:NB, 1] = gate_b

    # Toeplitz decay master: M[jl, m] = s_qk * decay^(128*(nd-1)+jl-m-1),
    # zero where the exponent would be negative (j <= i).
    jl = np.arange(P, dtype=np.float64)[:, None]
    mm = np.arange(MW, dtype=np.float64)[None, :]
    e = P * (nd - 1) + jl - mm - 1.0
    M = (s_qk * np.where(e >= 0, np.power(decay, np.maximum(e, 0.0)), 0.0)
         ).astype(BF)

    in_maps = []
    for c in range(8):
        b, qc = divmod(c, NQ)
        c0 = qc * QF
        navail = min(WIN, T - c0)
        win = np.zeros((WIN, K), np.float32)
        win[:navail] = g[b][c0:c0 + navail]
        gwc = np.ascontiguousarray(
            win.T.reshape(KT, P, WIN).transpose(1, 0, 2)).astype(BF)
        imap = {"gw": gwc, "m": M, "pack": pack, "small": small}
        if not const_gates:
            esel = np.zeros((NB, NB * P), np.float32)
            for _n in range(NB):
                esel[_n, _n * P:(_n + 1) * P] = s_base
            imap["esel"] = esel
        in_maps.append(imap)

    res = run_bass_kernel_spmd(nc, in_maps, list(range(8)))

    out = np.zeros((B, T, V), np.float32)
    for c in range(8):
        b, qc = divmod(c, NQ)
        oc = np.asarray(res.results[c]["o"], np.float32).reshape(QF, K)
        out[b, qc * QF:(qc + 1) * QF, :][:, write_idx] = oc
    return out
